# revision 4
# baseline (speedup 1.0000x reference)
"""Chamfer distance loss kernel for 8 Trainium2 NeuronCores.

Problem: template/source point clouds [B=4, N=8192, 3] fp32.
  d2[b,n,m] = ||t[b,n] - s[b,m]||^2
  out = mean_b( (mean_n sqrt(min_m d2) + mean_m sqrt(min_n d2)) / 2 )

Sharding: core c handles batch b=c//2, template-row half h=c%2.  Each
core computes its 4096x8192 slab of the distance matrix once and
extracts BOTH directions from it:
  - row minima (template->source): free-axis min per template row
  - column minima partials (source->template): running elementwise min
    across strips, partition-reduced at the end via PE transpose;
    the two cores sharing a batch are combined on the host.

Per-strip pipeline (strip = 128 template rows), all on PE + DVE:
  PE  : 16 matmuls [128,512] fill PSUM (two 4-bank groups,
        double-buffered).  K=14 f32r hi/lo split reproduces exact-fp32
        brackets b2[m] - 2 t.s.
  DVE : 4 tensor_scalar ops drain the PSUM groups directly
        (out = psum + a2[row] -> bf16 strip, measured ~3.5 elem/ns from
        PSUM on this silicon) with op1=min accumulators emitting one
        raw row-min partial per group ([128,1] fp32, a2 included); then
        one tensor_tensor min folds the strip into the running column
        min.  The last strip's fold writes fp32 directly so the
        epilogue needs no dtype-widening copy.

Measured op rates that drove this structure (chained, per op):
  TT min bf16 [128,8192] sbuf->sbuf    ~1.6 us
  TS +accum  [128,2048] psum->sbuf     ~0.6 us   (non-in-place!)
  TS +accum  in-place sbuf [128,8192]  ~6.6 us   (in-place forces slow
        mode - this was the old bottleneck)
  ACT        [128,2048] psum->sbuf     ~1.8 us   (3x slower than DVE
        at draining PSUM -> ACT removed from the hot loop entirely)

Row mins: 4 partials per strip land in out_row [128, 4*STRIPS]; the
host takes the min over the 4 groups, clamps, sqrts, and averages.
Column epilogue: fp32 colp is PE-transposed in 64 [128,128] blocks into
PSUM and min-reduced to [128,64]; host combines core pairs.
"""

import numpy as np

B = 4
N = 8192  # points per cloud
HALF = N // 2  # template rows per core
N_CORES = 8
STRIPS = HALF // 128  # 32
M_TILES = N // 512  # 16
K_ROWS = 14  # hi/lo-split contraction
CBLK = N // 128  # 64 column-min output blocks

_cache = {}


def _build_bass(reps=1, ablate=()):
    """ablate: subset of {'colp','drain','mm','epi'} to drop pieces
    for timing ablation (results are garbage when non-empty)."""
    import contextlib
    from concourse import bacc, mybir, tile, masks

    f32 = mybir.dt.float32
    f32r = mybir.dt.float32r
    bf16 = mybir.dt.bfloat16
    AOp = mybir.AluOpType

    nc = bacc.Bacc("TRN2", target_bir_lowering=False, debug=False,
                   num_devices=N_CORES)

    lhs = nc.dram_tensor("lhs", [K_ROWS, HALF], f32r,
                         kind="ExternalInput").ap()
    rhs = nc.dram_tensor("rhs", [K_ROWS, N], f32r,
                         kind="ExternalInput").ap()
    a2 = nc.dram_tensor("a2", [128, STRIPS], f32, kind="ExternalInput").ap()
    out_row = nc.dram_tensor("out_row", [128, 4 * STRIPS], f32,
                             kind="ExternalOutput").ap()
    out_col = nc.dram_tensor("out_col", [128, CBLK], f32,
                             kind="ExternalOutput").ap()

    with tile.TileContext(nc) as tc:
        with tc.tile_pool(name="const", bufs=1) as cpool, \
             tc.tile_pool(name="psum", bufs=1, space="PSUM") as ppool, \
             tc.tile_pool(name="strip", bufs=3) as spool:

            lhs_sb = cpool.tile([K_ROWS, HALF], f32r, tag="lhs")
            rhs_sb = cpool.tile([K_ROWS, N], f32r, tag="rhs")
            a2_sb = cpool.tile([128, STRIPS], f32, tag="a2")
            ident = cpool.tile([128, 128], f32, tag="ident")
            colp = cpool.tile([128, N], bf16, tag="colp")
            colpf = cpool.tile([128, N], f32, tag="colpf")
            rowacc = cpool.tile([128, 4 * STRIPS], f32, tag="rowacc")
            ocol_sb = cpool.tile([128, CBLK], f32, tag="ocol")

            nc.sync.dma_start(lhs_sb[:, :], lhs)
            nc.sync.dma_start(rhs_sb[:, :], rhs)
            nc.sync.dma_start(a2_sb[:, :], a2)
            masks.make_identity(nc, ident[:, :])

            # One PSUM tensor spanning all 8 banks, viewed [128, 32, 128]:
            # matmul tiles are 4 slots, drains cover 16 slots, and the
            # epilogue reuses slots 16..31 as transposed blocks.
            P = ppool.tile([128, 32, 128], f32, tag="P")

            loop_ctx = (tc.For_i(0, reps, 1) if reps > 1
                        else contextlib.nullcontext())
            with loop_ctx:
                for s in range(STRIPS):
                    strip_sb = spool.tile([128, N], bf16, tag="strip")
                    for g in range(4):
                        base = 16 * (g % 2)  # PSUM slot of this 4-bank group
                        if "mm" not in ablate:
                            for j in range(4):
                                m = 4 * g + j
                                nc.tensor.matmul(
                                    P[:, base + 4 * j: base + 4 * (j + 1), :],
                                    lhsT=lhs_sb[:, 128 * s: 128 * (s + 1)],
                                    rhs=rhs_sb[:, 512 * m: 512 * (m + 1)],
                                    start=True, stop=True,
                                )
                        if "drain" not in ablate:
                            # Drain + bias + row-min partial in one DVE op:
                            # out = psum + a2[row] (bf16), accum = min(out).
                            k = 4 * s + g
                            nc.vector.tensor_scalar(
                                strip_sb[:, 2048 * g: 2048 * (g + 1)],
                                P[:, base: base + 16, :],
                                a2_sb[:, s:s + 1], None,
                                AOp.add, AOp.min,
                                accum_out=rowacc[:, k:k + 1],
                            )
                    if "colp" not in ablate:
                        if s == 0:
                            nc.vector.tensor_copy(colp[:, :], strip_sb[:, :])
                        elif s == STRIPS - 1:
                            # final fold widens to fp32 for the transposes
                            nc.vector.tensor_tensor(
                                colpf[:, :], colp[:, :], strip_sb[:, :],
                                AOp.min,
                            )
                        else:
                            nc.vector.tensor_tensor(
                                colp[:, :], colp[:, :], strip_sb[:, :],
                                AOp.min,
                            )

                if "epi" not in ablate:
                    # column epilogue: partition-min via PE transpose (fp32)
                    # (only PSUM slots 16..31, so the next iteration's
                    # even-group matmuls don't stall behind the epilogue)
                    for r in range(4):
                        for t in range(16):
                            blk = 16 * r + t
                            nc.tensor.transpose(
                                P[:, 16 + t, :],
                                colpf[:, 128 * blk: 128 * (blk + 1)],
                                ident[:, :],
                            )
                        nc.vector.tensor_reduce(
                            ocol_sb[:, 16 * r: 16 * (r + 1)], P[:, 16:32, :],
                            axis=mybir.AxisListType.X, op=AOp.min,
                        )

                nc.sync.dma_start(out_row, rowacc[:, :])
                nc.sync.dma_start(out_col, ocol_sb[:, :])

    nc.compile()
    return nc


def _rnd11(x):
    """Round-to-nearest keeping 11 explicit mantissa bits (the rounding the
    PE applies to float32r operands, measured on HW)."""
    xi = x.view(np.uint32).astype(np.uint64)
    out = ((xi + np.uint64(1 << 11)) & np.uint64(0xFFFFF000)).astype(np.uint32)
    return out.view(np.float32)


def _hilo(x):
    hi = _rnd11(np.ascontiguousarray(x, np.float32))
    lo = _rnd11((x - hi).astype(np.float32))
    return hi, lo


def _sq(x):  # |x|^2 per point, fp32
    return (x * x).sum(axis=-1, dtype=np.float32)


def _prep_core_inputs(template, source, c):
    b, h = divmod(c, 2)
    tch = template[b, h * HALF:(h + 1) * HALF]  # [4096, 3] rows
    sfull = source[b]  # [8192, 3] cols

    # stationary operand: [14, 4096] = hi/lo split of -2*t
    v = (-2.0 * tch.T).astype(np.float32)  # [3, n]
    ones = np.ones((1, HALF), np.float32)
    vh, vl = _hilo(v)
    lhs = np.ascontiguousarray(
        np.concatenate([vh, vh, vl, vl, ones, ones], axis=0))

    # moving operand: [14, 8192] = hi/lo split of s plus |s|^2 rows
    w = np.ascontiguousarray(sfull.T, np.float32)  # [3, m]
    b2 = _sq(sfull)[None]  # [1, m]
    wh, wl = _hilo(w)
    b2h, b2l = _hilo(b2)
    rhs = np.ascontiguousarray(
        np.concatenate([wh, wl, wh, wl, b2h, b2l], axis=0))

    return {
        "lhs": lhs,
        "rhs": rhs,
        "a2": np.ascontiguousarray(_sq(tch).reshape(STRIPS, 128).T),
    }


def _run(template, source, trace=False):
    from concourse.bass_utils import run_bass_kernel_spmd

    template = np.asarray(template, np.float32)
    source = np.asarray(source, np.float32)
    assert template.shape == (B, N, 3) and source.shape == (B, N, 3)

    if "nc" not in _cache:
        _cache["nc"] = _build_bass()
    nc = _cache["nc"]

    in_maps = [_prep_core_inputs(template, source, c) for c in range(N_CORES)]
    res = run_bass_kernel_spmd(nc, in_maps, core_ids=list(range(N_CORES)),
                               trace=trace)

    rows = np.stack([np.asarray(r["out_row"], np.float64)
                     for r in res.results])  # [8, 128, 4*32] raw d2 partials
    cols = np.stack([np.asarray(r["out_col"], np.float64)
                     for r in res.results])  # [8, 128, 64]
    rowmin = rows.reshape(N_CORES, 128, STRIPS, 4).min(axis=3)  # [8,128,32]
    per_core_row = np.sqrt(np.maximum(rowmin, 0.0)).sum(axis=(1, 2))  # [8]
    cost01 = per_core_row.reshape(B, 2).sum(axis=1) / N  # [B]
    colmin = np.minimum(cols[0::2], cols[1::2])  # [B, 128, 64] raw d2
    cost10 = np.sqrt(np.maximum(colmin, 0.0)).mean(axis=(1, 2))  # [B]
    chamfer = ((cost01 + cost10) / 2.0).mean()
    return np.asarray(chamfer, dtype=np.float32), res


def kernel(template, source):
    val, _ = _run(template, source, trace=False)
    return val


# revision 10
# speedup vs baseline: 1.0959x; 1.0959x over previous
"""Chamfer distance loss kernel for 8 Trainium2 NeuronCores.

Problem: template/source point clouds [B=4, N=8192, 3] fp32.
  d2[b,n,m] = ||t[b,n] - s[b,m]||^2
  out = mean_b( (mean_n sqrt(min_m d2) + mean_m sqrt(min_n d2)) / 2 )

Sharding: core c handles batch b=c//2, template-row half h=c%2.  Each
core computes its 4096x8192 slab of the distance matrix once and
extracts BOTH directions from it:
  - row minima (template->source): free-axis min per template row
  - column minima partials (source->template): running elementwise min
    across strips, partition-reduced at the end via PE transpose;
    the two cores sharing a batch are combined on the host.

Per-strip pipeline (strip = 128 template rows), all on PE + DVE:
  PE  : 16 matmuls [128,512] fill PSUM (two 4-bank groups,
        double-buffered).  K=14 f32r hi/lo split reproduces exact-fp32
        brackets b2[m] - 2 t.s.
  DVE : 4 tensor_scalar ops drain the PSUM groups directly
        (out = psum + a2[row] -> bf16 strip, measured ~3.5 elem/ns from
        PSUM on this silicon) with op1=min accumulators emitting one
        raw row-min partial per group ([128,1] fp32, a2 included); then
        one tensor_tensor min folds the strip into the running column
        min.  The last strip's fold writes fp32 directly so the
        epilogue needs no dtype-widening copy.

Measured op rates that drove this structure (chained, per op):
  TT min bf16 [128,8192] sbuf->sbuf    ~1.6 us
  TS +accum  [128,2048] psum->sbuf     ~0.6 us   (non-in-place!)
  TS +accum  in-place sbuf [128,8192]  ~6.6 us   (in-place forces slow
        mode - this was the old bottleneck)
  ACT        [128,2048] psum->sbuf     ~1.8 us   (3x slower than DVE
        at draining PSUM -> ACT removed from the hot loop entirely)

Row mins: 4 partials per strip land in out_row [128, 4*STRIPS]; the
host takes the min over the 4 groups, clamps, sqrts, and averages.
Column epilogue: fp32 colp is PE-transposed in 64 [128,128] blocks into
PSUM and min-reduced to [128,64]; host combines core pairs.
"""

import numpy as np

B = 4
N = 8192  # points per cloud
HALF = N // 2  # template rows per core
N_CORES = 8
STRIPS = HALF // 128  # 32
M_TILES = N // 512  # 16
K_ROWS = 16  # hi/lo-split contraction (incl. a2 rows)
CBLK = N // 128  # 64 column-min output blocks

_cache = {}


def _build_bass(reps=1, ablate=()):
    """ablate: subset of {'colp','drain','mm','epi'} to drop pieces
    for timing ablation (results are garbage when non-empty)."""
    import contextlib
    from concourse import bacc, mybir, tile, masks

    f32 = mybir.dt.float32
    f32r = mybir.dt.float32r
    bf16 = mybir.dt.bfloat16
    AOp = mybir.AluOpType

    nc = bacc.Bacc("TRN2", target_bir_lowering=False, debug=False,
                   num_devices=N_CORES)

    lhs = nc.dram_tensor("lhs", [K_ROWS, HALF], f32r,
                         kind="ExternalInput").ap()
    rhs = nc.dram_tensor("rhs", [K_ROWS, N], f32r,
                         kind="ExternalInput").ap()
    out_row = nc.dram_tensor("out_row", [128, 4 * STRIPS], f32,
                             kind="ExternalOutput").ap()
    out_col = nc.dram_tensor("out_col", [128, CBLK], f32,
                             kind="ExternalOutput").ap()

    with tile.TileContext(nc) as tc:
        with tc.tile_pool(name="const", bufs=1) as cpool, \
             tc.tile_pool(name="psum", bufs=1, space="PSUM") as ppool, \
             tc.tile_pool(name="strip", bufs=3) as spool:

            lhs_sb = cpool.tile([K_ROWS, HALF], f32r, tag="lhs")
            rhs_sb = cpool.tile([K_ROWS, N], f32r, tag="rhs")
            ident = cpool.tile([128, 128], f32, tag="ident")
            colp = cpool.tile([128, N], bf16, tag="colp")
            colpf = cpool.tile([128, N], f32, tag="colpf")
            rowacc = cpool.tile([128, 4 * STRIPS], f32, tag="rowacc")
            ocol_sb = cpool.tile([128, CBLK], f32, tag="ocol")

            nc.sync.dma_start(lhs_sb[:, :], lhs)
            nc.sync.dma_start(rhs_sb[:, :], rhs)
            masks.make_identity(nc, ident[:, :])

            # One PSUM tensor spanning all 8 banks, viewed [128, 32, 128]:
            # matmul tiles are 4 slots, drains cover 16 slots, and the
            # epilogue reuses slots 16..31 as transposed blocks.
            P = ppool.tile([128, 32, 128], f32, tag="P")

            loop_ctx = (tc.For_i(0, reps, 1) if reps > 1
                        else contextlib.nullcontext())
            with loop_ctx:
                for s in range(STRIPS):
                    strip_sb = spool.tile([128, N], bf16, tag="strip")
                    for g in range(4):
                        base = 16 * (g % 2)  # PSUM slot of this 4-bank group
                        if "mm" not in ablate:
                            for j in range(4):
                                m = 4 * g + j
                                nc.tensor.matmul(
                                    P[:, base + 4 * j: base + 4 * (j + 1), :],
                                    lhsT=lhs_sb[:, 128 * s: 128 * (s + 1)],
                                    rhs=rhs_sb[:, 512 * m: 512 * (m + 1)],
                                    start=True, stop=True,
                                )
                        if "drain" not in ablate:
                            # Drain + row-min partial in one DVE op: the
                            # full d2 (incl. a2, added by the PE via the
                            # a2*ones contraction rows) copies to bf16
                            # strip, accum = min(out).  The scalar must be
                            # an immediate: an AP scalar lowers to the slow
                            # TensorScalarPtr path (measured ~5x).
                            k = 4 * s + g
                            nc.vector.tensor_scalar(
                                strip_sb[:, 2048 * g: 2048 * (g + 1)],
                                P[:, base: base + 16, :],
                                3.0e38, None,
                                AOp.min, AOp.min,
                                accum_out=rowacc[:, k:k + 1],
                            )
                    if "colp" not in ablate:
                        if s == 0:
                            nc.vector.tensor_copy(colp[:, :], strip_sb[:, :])
                        elif s == STRIPS - 1:
                            # final fold widens to fp32 for the transposes
                            nc.vector.tensor_tensor(
                                colpf[:, :], colp[:, :], strip_sb[:, :],
                                AOp.min,
                            )
                        else:
                            nc.vector.tensor_tensor(
                                colp[:, :], colp[:, :], strip_sb[:, :],
                                AOp.min,
                            )

                if "epi" not in ablate:
                    # column epilogue: partition-min via PE transpose (fp32)
                    # (only PSUM slots 16..31, so the next iteration's
                    # even-group matmuls don't stall behind the epilogue)
                    for r in range(4):
                        for t in range(16):
                            blk = 16 * r + t
                            nc.tensor.transpose(
                                P[:, 16 + t, :],
                                colpf[:, 128 * blk: 128 * (blk + 1)],
                                ident[:, :],
                            )
                        nc.vector.tensor_reduce(
                            ocol_sb[:, 16 * r: 16 * (r + 1)], P[:, 16:32, :],
                            axis=mybir.AxisListType.X, op=AOp.min,
                        )

                nc.sync.dma_start(out_row, rowacc[:, :])
                nc.sync.dma_start(out_col, ocol_sb[:, :])

    nc.compile()
    return nc


def _rnd11(x):
    """Round-to-nearest keeping 11 explicit mantissa bits (the rounding the
    PE applies to float32r operands, measured on HW)."""
    xi = x.view(np.uint32).astype(np.uint64)
    out = ((xi + np.uint64(1 << 11)) & np.uint64(0xFFFFF000)).astype(np.uint32)
    return out.view(np.float32)


def _hilo(x):
    hi = _rnd11(np.ascontiguousarray(x, np.float32))
    lo = _rnd11((x - hi).astype(np.float32))
    return hi, lo


def _sq(x):  # |x|^2 per point, fp32
    return (x * x).sum(axis=-1, dtype=np.float32)


def _prep_core_inputs(template, source, c):
    b, h = divmod(c, 2)
    tch = template[b, h * HALF:(h + 1) * HALF]  # [4096, 3] rows
    sfull = source[b]  # [8192, 3] cols

    # stationary operand: [16, 4096] = hi/lo split of -2*t, ones rows for
    # the |s|^2 bracket, and hi/lo |t|^2 rows (paired with ones on the
    # moving side) so the PE emits the complete d2 with no bias pass.
    v = (-2.0 * tch.T).astype(np.float32)  # [3, n]
    ones_l = np.ones((1, HALF), np.float32)
    vh, vl = _hilo(v)
    a2h, a2l = _hilo(_sq(tch)[None])  # [1, n]
    lhs = np.ascontiguousarray(
        np.concatenate([vh, vh, vl, vl, ones_l, ones_l, a2h, a2l], axis=0))

    # moving operand: [16, 8192] = hi/lo split of s, |s|^2 rows, ones rows
    w = np.ascontiguousarray(sfull.T, np.float32)  # [3, m]
    b2 = _sq(sfull)[None]  # [1, m]
    wh, wl = _hilo(w)
    b2h, b2l = _hilo(b2)
    ones_r = np.ones((1, N), np.float32)
    rhs = np.ascontiguousarray(
        np.concatenate([wh, wl, wh, wl, b2h, b2l, ones_r, ones_r], axis=0))

    return {"lhs": lhs, "rhs": rhs}


def _run(template, source, trace=False):
    from concourse.bass_utils import run_bass_kernel_spmd

    template = np.asarray(template, np.float32)
    source = np.asarray(source, np.float32)
    assert template.shape == (B, N, 3) and source.shape == (B, N, 3)

    if "nc" not in _cache:
        _cache["nc"] = _build_bass()
    nc = _cache["nc"]

    in_maps = [_prep_core_inputs(template, source, c) for c in range(N_CORES)]
    res = run_bass_kernel_spmd(nc, in_maps, core_ids=list(range(N_CORES)),
                               trace=trace)

    rows = np.stack([np.asarray(r["out_row"], np.float64)
                     for r in res.results])  # [8, 128, 4*32] raw d2 partials
    cols = np.stack([np.asarray(r["out_col"], np.float64)
                     for r in res.results])  # [8, 128, 64]
    rowmin = rows.reshape(N_CORES, 128, STRIPS, 4).min(axis=3)  # [8,128,32]
    per_core_row = np.sqrt(np.maximum(rowmin, 0.0)).sum(axis=(1, 2))  # [8]
    cost01 = per_core_row.reshape(B, 2).sum(axis=1) / N  # [B]
    colmin = np.minimum(cols[0::2], cols[1::2])  # [B, 128, 64] raw d2
    cost10 = np.sqrt(np.maximum(colmin, 0.0)).mean(axis=(1, 2))  # [B]
    chamfer = ((cost01 + cost10) / 2.0).mean()
    return np.asarray(chamfer, dtype=np.float32), res


def kernel(template, source):
    val, _ = _run(template, source, trace=False)
    return val


# revision 21
# speedup vs baseline: 1.3289x; 1.2126x over previous
"""Chamfer distance loss kernel for 8 Trainium2 NeuronCores.

Problem: template/source point clouds [B=4, N=8192, 3] fp32.
  d2[b,n,m] = ||t[b,n] - s[b,m]||^2
  out = mean_b( (mean_n sqrt(min_m d2) + mean_m sqrt(min_n d2)) / 2 )

Sharding: core c handles batch b=c//2, template-row half h=c%2.  Each
core computes its 4096x8192 slab of the distance matrix once and
extracts BOTH directions from it:
  - row minima (template->source): free-axis min per template row
  - column minima partials (source->template): running elementwise min
    across strips, partition-reduced at the end via PE transpose;
    the two cores sharing a batch are combined on the host.

Per-strip pipeline (strip = 128 template rows), all on PE + DVE:
  PE  : 16 matmuls [128,512] fill PSUM (two 4-bank groups,
        double-buffered).  K=14 f32r hi/lo split reproduces exact-fp32
        brackets b2[m] - 2 t.s.
  DVE : 4 tensor_scalar ops drain the PSUM groups directly
        (out = psum + a2[row] -> bf16 strip, measured ~3.5 elem/ns from
        PSUM on this silicon) with op1=min accumulators emitting one
        raw row-min partial per group ([128,1] fp32, a2 included); then
        one tensor_tensor min folds the strip into the running column
        min.  The last strip's fold writes fp32 directly so the
        epilogue needs no dtype-widening copy.

Measured op rates that drove this structure (chained, per op):
  TT min bf16 [128,8192] sbuf->sbuf    ~1.6 us
  TS +accum  [128,2048] psum->sbuf     ~0.6 us   (non-in-place!)
  TS +accum  in-place sbuf [128,8192]  ~6.6 us   (in-place forces slow
        mode - this was the old bottleneck)
  ACT        [128,2048] psum->sbuf     ~1.8 us   (3x slower than DVE
        at draining PSUM -> ACT removed from the hot loop entirely)

Row mins: 4 partials per strip land in out_row [128, 4*STRIPS]; the
host takes the min over the 4 groups, clamps, sqrts, and averages.
Column epilogue: fp32 colp is PE-transposed in 64 [128,128] blocks into
PSUM and min-reduced to [128,64]; host combines core pairs.
"""

import numpy as np

B = 4
N = 8192  # points per cloud
HALF = N // 2  # template rows per core
N_CORES = 8
STRIPS = HALF // 128  # 32
M_TILES = N // 512  # 16
K_ROWS = 24  # bf16 triple-split contraction (incl. b2 and a2 rows)
CBLK = N // 128  # 64 column-min output blocks

_cache = {}


def _build_bass(reps=1, ablate=()):
    """ablate: subset of {'colp','drain','mm','epi'} to drop pieces
    for timing ablation (results are garbage when non-empty)."""
    import contextlib
    from concourse import bacc, mybir, tile, masks

    f32 = mybir.dt.float32
    f32r = mybir.dt.float32r
    bf16 = mybir.dt.bfloat16
    AOp = mybir.AluOpType

    ablate = set(ablate)
    if "drain" in ablate:
        ablate |= {"colp"}
    if "colp" in ablate:
        ablate |= {"epi"}

    nc = bacc.Bacc("TRN2", target_bir_lowering=False, debug=False,
                   num_devices=N_CORES)

    lhs = nc.dram_tensor("lhs", [K_ROWS, HALF], bf16,
                         kind="ExternalInput").ap()
    rhs = nc.dram_tensor("rhs", [K_ROWS, N], bf16,
                         kind="ExternalInput").ap()
    out_row = nc.dram_tensor("out_row", [128, 4 * STRIPS], f32,
                             kind="ExternalOutput").ap()
    out_col = nc.dram_tensor("out_col", [128, CBLK], f32,
                             kind="ExternalOutput").ap()

    with tile.TileContext(nc) as tc:
        with tc.tile_pool(name="const", bufs=1) as cpool, \
             tc.tile_pool(name="psum", bufs=1, space="PSUM") as ppool:

            lhs_sb = cpool.tile([K_ROWS, HALF], bf16, tag="lhs")
            rhs_sb = cpool.tile([K_ROWS, N], bf16, tag="rhs")
            strip_sb = cpool.tile([128, N], bf16, tag="stripbuf")
            ident = cpool.tile([128, 128], f32, tag="ident")
            colp = cpool.tile([128, N], bf16, tag="colp")
            colpf = cpool.tile([128, N], f32, tag="colpf")
            rowacc = cpool.tile([128, 4 * STRIPS], f32, tag="rowacc")
            ocol_sb = cpool.tile([128, CBLK], f32, tag="ocol")

            nc.sync.dma_start(lhs_sb[:, :], lhs)
            nc.sync.dma_start(rhs_sb[:, :], rhs)
            masks.make_identity(nc, ident[:, :])
            if ablate:
                # keep every output/read defined under any ablation combo
                nc.vector.memset(rowacc[:, :], 0.0)
                nc.vector.memset(ocol_sb[:, :], 0.0)
                nc.vector.memset(colpf[:, :], 0.0)
                nc.vector.memset(colp[:, :], 0.0)

            # One PSUM tensor spanning all 8 banks, viewed [128, 32, 128]:
            # matmul tiles are 4 slots, drains cover 16 slots, and the
            # epilogue reuses slots 16..31 as transposed blocks.
            P = ppool.tile([128, 32, 128], f32, tag="P")
            if "mm" in ablate and "drain" not in ablate:
                nc.vector.memset(P[:, :, :], 0.0)

            loop_ctx = (tc.For_i(0, reps, 1) if reps > 1
                        else contextlib.nullcontext())
            with loop_ctx:
                # Single fixed strip buffer: the DVE consumes strips in
                # issue order (drains then fold), so rotation buys no
                # pipelining, and per-strip tile alloc/release was measured
                # at ~2.7us/strip of pure overhead.
                for s in range(STRIPS):
                    for g in range(4):
                        base = 16 * (g % 2)  # PSUM slot of this 4-bank group
                        if "mm" not in ablate:
                            for j in range(4):
                                m = 4 * g + j
                                nc.tensor.matmul(
                                    P[:, base + 4 * j: base + 4 * (j + 1), :],
                                    lhsT=lhs_sb[:, 128 * s: 128 * (s + 1)],
                                    rhs=rhs_sb[:, 512 * m: 512 * (m + 1)],
                                    start=True, stop=True,
                                )
                        if "drain" not in ablate:
                            # Drain + row-min partial in one DVE op: the
                            # full d2 (incl. a2, added by the PE via the
                            # a2*ones contraction rows) copies to bf16
                            # strip, accum = min(out).  The scalar must be
                            # an immediate: an AP scalar lowers to the slow
                            # TensorScalarPtr path (measured ~5x).
                            k = 4 * s + g
                            nc.vector.tensor_scalar(
                                strip_sb[:, 2048 * g: 2048 * (g + 1)],
                                P[:, base: base + 16, :],
                                3.0e38, None,
                                AOp.min, AOp.min,
                                accum_out=rowacc[:, k:k + 1],
                            )
                    if "colp" not in ablate:
                        if s == 0:
                            nc.vector.tensor_copy(colp[:, :], strip_sb[:, :])
                        elif s == STRIPS - 1:
                            # final fold widens to fp32 for the transposes
                            nc.vector.tensor_tensor(
                                colpf[:, :], colp[:, :], strip_sb[:, :],
                                AOp.min,
                            )
                        else:
                            nc.vector.tensor_tensor(
                                colp[:, :], colp[:, :], strip_sb[:, :],
                                AOp.min,
                            )

                if "epi" not in ablate:
                    # column epilogue: partition-min via PE transpose (fp32)
                    # (only PSUM slots 16..31, so the next iteration's
                    # even-group matmuls don't stall behind the epilogue)
                    for r in range(4):
                        for t in range(16):
                            blk = 16 * r + t
                            nc.tensor.transpose(
                                P[:, 16 + t, :],
                                colpf[:, 128 * blk: 128 * (blk + 1)],
                                ident[:, :],
                            )
                        nc.vector.tensor_reduce(
                            ocol_sb[:, 16 * r: 16 * (r + 1)], P[:, 16:32, :],
                            axis=mybir.AxisListType.X, op=AOp.min,
                        )

                nc.sync.dma_start(out_row, rowacc[:, :])
                nc.sync.dma_start(out_col, ocol_sb[:, :])

    nc.compile()
    return nc


def _split3(x):
    """Exact-ish triple bf16 split: x ~= h + m + l with ~24 mantissa bits."""
    import ml_dtypes
    bf = ml_dtypes.bfloat16
    x = np.ascontiguousarray(x, np.float32)
    h = x.astype(bf)
    r = (x - h.astype(np.float32)).astype(np.float32)
    m = r.astype(bf)
    l = (r - m.astype(np.float32)).astype(bf)
    return h, m, l


def _sq(x):  # |x|^2 per point, fp32
    return (x * x).sum(axis=-1, dtype=np.float32)


def _prep_core_inputs(template, source, c):
    b, h = divmod(c, 2)
    tch = template[b, h * HALF:(h + 1) * HALF]  # [4096, 3] rows
    sfull = source[b]  # [8192, 3] cols

    # Triple bf16 split emulating fp32: v.w ~= vh(wh+wm+wl) + vm(wh+wm)
    # + vl.wh, dropping O(2^-27) cross terms.  b2 (|s|^2) rides ones rows
    # on the stationary side; a2 (|t|^2) rides ones rows on the moving
    # side, so the PE emits the complete d2 with no bias pass.
    import ml_dtypes
    bf = ml_dtypes.bfloat16
    v = (-2.0 * tch.T).astype(np.float32)  # [3, n]
    ones_l = np.ones((1, HALF), bf)
    vh, vm, vl = _split3(v)
    a2h, a2m, a2l = _split3(_sq(tch)[None])  # [1, n]
    lhs = np.ascontiguousarray(np.concatenate(
        [vh, vh, vh, vm, vm, vl,
         ones_l, ones_l, ones_l, a2h, a2m, a2l], axis=0))

    w = np.ascontiguousarray(sfull.T, np.float32)  # [3, m]
    b2 = _sq(sfull)[None]  # [1, m]
    wh, wm, wl = _split3(w)
    b2h, b2m, b2l = _split3(b2)
    ones_r = np.ones((1, N), bf)
    rhs = np.ascontiguousarray(np.concatenate(
        [wh, wm, wl, wh, wm, wh,
         b2h, b2m, b2l, ones_r, ones_r, ones_r], axis=0))

    return {"lhs": lhs, "rhs": rhs}


def _run(template, source, trace=False):
    from concourse.bass_utils import run_bass_kernel_spmd

    template = np.asarray(template, np.float32)
    source = np.asarray(source, np.float32)
    assert template.shape == (B, N, 3) and source.shape == (B, N, 3)

    if "nc" not in _cache:
        _cache["nc"] = _build_bass()
    nc = _cache["nc"]

    in_maps = [_prep_core_inputs(template, source, c) for c in range(N_CORES)]
    res = run_bass_kernel_spmd(nc, in_maps, core_ids=list(range(N_CORES)),
                               trace=trace)

    rows = np.stack([np.asarray(r["out_row"], np.float64)
                     for r in res.results])  # [8, 128, 4*32] raw d2 partials
    cols = np.stack([np.asarray(r["out_col"], np.float64)
                     for r in res.results])  # [8, 128, 64]
    rowmin = rows.reshape(N_CORES, 128, STRIPS, 4).min(axis=3)  # [8,128,32]
    per_core_row = np.sqrt(np.maximum(rowmin, 0.0)).sum(axis=(1, 2))  # [8]
    cost01 = per_core_row.reshape(B, 2).sum(axis=1) / N  # [B]
    colmin = np.minimum(cols[0::2], cols[1::2])  # [B, 128, 64] raw d2
    cost10 = np.sqrt(np.maximum(colmin, 0.0)).mean(axis=(1, 2))  # [B]
    chamfer = ((cost01 + cost10) / 2.0).mean()
    return np.asarray(chamfer, dtype=np.float32), res


def kernel(template, source):
    val, _ = _run(template, source, trace=False)
    return val
